# revision 13
# baseline (speedup 1.0000x reference)
"""BERT-CRF loss kernel for Trainium2 (8 NeuronCores, data-parallel over batch).

Computation: emissions = x @ W.T + b; CRF NLL = mean over batch of
(denominator log-partition - numerator tag-path score).

v3 strategy per core (2 sequences, 8192 time steps):
  Sharding/layout: each core receives its batch shard pre-transposed as
  xT [768, 8192] (h-major), so the h-contraction lands on the partition dim
  directly -- no on-device transposes or casts.

  Stage 1 (memory-bound bulk): 8 big DMAs ([128, 6, 1024] f32 = 3 MB each,
  issued alternately from the SP and ACT queues) stream xT through SBUF;
  per 512-t group, 6 accumulating f32r matmuls produce e[3, 512] in PSUM at
  full moving-rate (f32r streams 1 col/cycle for >=256-col moves); PSUM is
  staged to SBUF and redistributed by 3 small DMAs per group into per-half
  tiles e_sb[h][p, c, u] (partition p holds 64 consecutive time steps).

  Stage 2 (CRF denominator): forward algorithm as a chain of log-semiring
  products of 3x3 matrices M_t[i,j] = trans[i,j] + b[j] + e_t[j].  Each
  partition tree-combines its 64 consecutive matrices in 3 levels (64 -> 8),
  using a host-precomputed K[i,k,j] = ct[i,j] + ct[j,k] table to fuse level 1
  and a per-timestep-max rescaling so level 1 needs no max-reduce.  The half
  covering sequence 0 runs interleaved with stage-1 groups 8-15; only the
  second half's tree is a tail.  The remaining 8 matrices per partition ship
  to the host, which finishes the product per sequence in float64.

  Numerator: e * one-hot(y) multiply + free-dim reduce per half gives
  sum_t e[t, y_t] per partition; host sums and adds start/end/transition/bias
  path scores (tiny O(B*S) int gathers, as in torchcrf's score decomposition).

Assumes mask == all-ones (guaranteed by the problem spec: fill "ones").
"""

import sys

sys.path.insert(0, "/opt/trn_rl_repo")

import numpy as np
from contextlib import ExitStack

import concourse.bass as bass
import concourse.mybir as mybir
import concourse.tile as tile
from concourse.bass_utils import run_bass_kernel_spmd

dt = mybir.dt
AF = mybir.ActivationFunctionType
ALU = mybir.AluOpType
AX = mybir.AxisListType

# ---------------------------------------------------------------------------
# The walrus build in this container accepts at most ONE sync wait per
# instruction.  Legalize the serialized BIR by moving extra waits onto
# preceding same-engine NoOps (each carrying exactly one wait).
# ---------------------------------------------------------------------------
_orig_to_json_bytes = bass.Bass.to_json_bytes


def _legalized_to_json_bytes(self):
    import json as _json

    m = _json.loads(_orig_to_json_bytes(self))
    ctr = 0
    for fn in m.get("functions", []):
        for blk in fn.get("blocks", []):
            insts = blk.get("instructions", [])
            out = []
            for inst in insts:
                si = inst.get("sync_info") or {}
                waits = si.get("on_wait") or []
                if len(waits) > 1:
                    for w in waits[:-1]:
                        ctr += 1
                        out.append(
                            {
                                "debug": inst.get("debug", 0),
                                "engine": inst["engine"],
                                "ins": [],
                                "outs": [],
                                "name": f"lw-{ctr}",
                                "opcode": "NoOp",
                                "sync_info": {"on_update": [], "on_wait": [w]},
                            }
                        )
                    si["on_wait"] = [waits[-1]]
                out.append(inst)
            blk["instructions"] = out
    return _json.dumps(m).encode()


bass.Bass.to_json_bytes = _legalized_to_json_bytes

B, S, H, T = 16, 4096, 768, 3
NCORES = 8
BL = B // NCORES          # sequences per core = 2
NT = BL * S               # 8192 time steps per core
NGROUP = 16               # groups of 512 time steps
NBLOCK = 8                # xT DMA blocks of 2 groups (1024 t)
HC = H // 128             # 6 h-chunks
UP = 64                   # time steps per partition
L3M = 8                   # matrices per partition shipped to host

_CACHE = {}


def _emit_combine(nc, scr, half, lvl, nm, c_in, c_out):
    """Generic log-semiring pair-combine level: c_in [64, 2*nm, 9] (as flat
    [64, 18*nm]) -> c_out view [64, nm*9].  9 ops: 3 S-adds, maxred, sub,
    exp(ACT), addred, ln(ACT), add."""
    s_t = scr.tile([64, nm * 27], dt.float32, tag=f"s{lvl}", name=f"s{lvl}_{half}")
    sub_t = scr.tile([64, nm * 27], dt.float32, tag=f"sb{lvl}", name=f"sb{lvl}_{half}")
    mx_t = scr.tile([64, nm * 9], dt.float32, tag=f"mx{lvl}", name=f"mx{lvl}_{half}")
    sm_t = scr.tile([64, nm * 9], dt.float32, tag=f"sm{lvl}", name=f"sm{lvl}_{half}")
    cv = c_in[:].rearrange("p (m two e) -> p m two e", two=2, e=9)
    b_kj = cv[:, :, 1, :].rearrange("p m (j k) -> p m k j", k=3)          # [p,m,k,j]
    s5 = s_t[:].rearrange("p (m i k j) -> p m i k j", i=3, k=3, j=3)
    for i in range(3):
        a_i = (
            cv[:, :, 0, 3 * i : 3 * i + 3]
            .unsqueeze(2)
            .broadcast_to([64, nm, 3, 3])
        )                                                                  # [p,m,k0,j]
        nc.vector.tensor_tensor(s5[:, :, i, :, :], a_i, b_kj, op=ALU.add)
    s3 = s_t[:].rearrange("p (g j) -> p g j", j=3)
    nc.vector.tensor_reduce(mx_t[:], s3, axis=AX.X, op=ALU.max)
    mx_b = mx_t[:].unsqueeze(2).broadcast_to([64, nm * 9, 3])
    sub3 = sub_t[:].rearrange("p (g j) -> p g j", j=3)
    nc.vector.tensor_tensor(sub3, s3, mx_b, op=ALU.subtract)
    nc.scalar.activation(sub_t[:], sub_t[:], AF.Exp)
    nc.vector.tensor_reduce(sm_t[:], sub3, axis=AX.X, op=ALU.add)
    nc.scalar.activation(sm_t[:], sm_t[:], AF.Ln)
    nc.vector.tensor_tensor(c_out, sm_t[:], mx_t[:], op=ALU.add)


def _build_program():
    nc = bass.Bass()
    tc = tile.TileContext(nc)

    # ---- DRAM I/O ----
    xt_d = nc.dram_tensor("xt", [H, NT], dt.float32r, kind="ExternalInput")
    wt_d = nc.dram_tensor("wt", [128, HC * T], dt.float32r, kind="ExternalInput")
    cf_d = nc.dram_tensor("cf", [64, 54 + 2 * 192], dt.float32, kind="ExternalInput")
    op_d = nc.dram_tensor("op", [64, 2 * L3M * 9 + 2], dt.float32, kind="ExternalOutput")

    with tc, ExitStack() as ctx:
        const_pool = ctx.enter_context(tc.tile_pool(name="const", bufs=1))
        xg_pool = ctx.enter_context(tc.tile_pool(name="xg", bufs=2))
        st_pool = ctx.enter_context(tc.tile_pool(name="st", bufs=2))
        e_pool = ctx.enter_context(tc.tile_pool(name="e", bufs=1))
        scr_pool = ctx.enter_context(tc.tile_pool(name="scr", bufs=1))
        ps_e_pool = ctx.enter_context(tc.tile_pool(name="pse", bufs=2, space="PSUM"))

        # ---- constants (issued after the first xT block DMA) ----
        wt_sb = const_pool.tile([128, HC * T], dt.float32r, tag="wt")
        cf_sb = const_pool.tile([64, 54 + 2 * 192], dt.float32, tag="cf")
        k1_v = cf_sb[:, 0:27].rearrange("p (ik j) -> p ik j", j=3)
        k0_v = cf_sb[:, 27:54].rearrange("p (ik j) -> p ik j", j=3)

        # per-half emission tiles: e_sb[h][p, c, u], partition p holds
        # 64 consecutive time steps of sequence h
        e_sb = [
            e_pool.tile([64, T * UP], dt.float32, tag=f"e{h}", name=f"e{h}")
            for h in range(2)
        ]
        # outputs staging: 2*72 tree results + 2 numerator columns
        op_st = e_pool.tile([64, 2 * L3M * 9 + 2], dt.float32, tag="opst")

        def emit_tree(half, part, plo=0, phi=64):
            """Emit one chunk of the in-partition tree for one half.
            part 0: rescale prep + L1 S-build; part 1: L1 finish;
            part 2: L2; part 3: L3 + write into op_st.  Parts 0/1 may be
            emitted for a partition slice [plo:phi] to overlap stage 1."""
            np_ = phi - plo
            e3 = e_sb[half][:].rearrange("p (c u) -> p c u", u=UP)[plo:phi]
            if part == 0:
                if plo == 0:
                    _CACHE[f"tree{half}"] = (
                        scr_pool.tile([64, UP], dt.float32, tag="emax",
                                      name=f"emax{half}"),
                        scr_pool.tile([64, T * UP], dt.float32, tag="es",
                                      name=f"es{half}"),
                        scr_pool.tile([64, 32 * 27], dt.float32, tag="s1",
                                      name=f"s1_{half}"),
                    )
                emax, es_t, s1 = _CACHE[f"tree{half}"]
                emx = emax[plo:phi]
                # emax[p,u] = max_c e[p,c,u];  es = e - emax (range <= 0)
                nc.vector.tensor_tensor(
                    emx, e3[:, 0, :], e3[:, 1, :], op=ALU.max
                )
                nc.vector.tensor_tensor(
                    emx, emx, e3[:, 2, :], op=ALU.max
                )
                emax_b = emx.unsqueeze(1).broadcast_to([np_, T, UP])
                nc.vector.tensor_tensor(
                    es_t[:].rearrange("p (c u) -> p c u", u=UP)[plo:phi], e3,
                    emax_b, op=ALU.subtract,
                )
                # L1 S-build: S[p,m,ik,j] = K[ik,j] + esA[m,j]
                es3 = es_t[:].rearrange("p (c u) -> p c u", u=UP)[plo:phi]
                esA = es3.rearrange("p c (m two) -> p m two c", two=2)
                s4 = s1[:].rearrange("p (m ik j) -> p m ik j", ik=9, j=3)[plo:phi]
                a1 = esA[:, 1:, 0, :].unsqueeze(2).broadcast_to([np_, 31, 9, 3])
                nc.vector.tensor_tensor(
                    s4[:, 1:, :, :],
                    k1_v[plo:phi].unsqueeze(1).broadcast_to([np_, 31, 9, 3]),
                    a1, op=ALU.add,
                )
                a0 = esA[:, 0:1, 0, :].unsqueeze(2).broadcast_to([np_, 1, 9, 3])
                nc.vector.tensor_tensor(
                    s4[:, 0:1, :, :],
                    k0_v[plo:phi].unsqueeze(1).broadcast_to([np_, 1, 9, 3]),
                    a0, op=ALU.add,
                )
            elif part == 1:
                emax, es_t, s1 = _CACHE[f"tree{half}"]
                es3 = es_t[:].rearrange("p (c u) -> p c u", u=UP)[plo:phi]
                if plo == 0:
                    _CACHE[f"tree{half}b"] = (
                        scr_pool.tile([64, 288], dt.float32, tag="sm1",
                                      name=f"sm1_{half}"),
                        scr_pool.tile([64, 288], dt.float32, tag="c1",
                                      name=f"c1_{half}"),
                        scr_pool.tile([64, 32], dt.float32, tag="ems",
                                      name=f"ems{half}"),
                    )
                sm1, c1, emsum = _CACHE[f"tree{half}b"]
                nc.scalar.activation(s1[plo:phi], s1[plo:phi], AF.Exp)
                nc.vector.tensor_reduce(
                    sm1[plo:phi],
                    s1[:].rearrange("p (g j) -> p g j", j=3)[plo:phi],
                    axis=AX.X, op=ALU.add,
                )
                nc.scalar.activation(sm1[plo:phi], sm1[plo:phi], AF.Ln)
                # C1 = ln-sum + esB[k] + (emaxA + emaxB)  (= lnsum + eB + emaxA)
                esB = (
                    es3.rearrange("p c (m two) -> p m two c", two=2)[:, :, 1, :]
                    .unsqueeze(2)
                    .broadcast_to([np_, 32, 3, 3])
                )                                                  # [p,m,i0,k]
                c14 = c1[:].rearrange("p (m i k) -> p m i k", i=3, k=3)[plo:phi]
                nc.vector.tensor_tensor(
                    c14,
                    sm1[:].rearrange("p (m i k) -> p m i k", i=3, k=3)[plo:phi],
                    esB, op=ALU.add,
                )
                em2 = emax[:].rearrange("p (m two) -> p m two", two=2)[plo:phi]
                nc.vector.tensor_tensor(
                    emsum[plo:phi], em2[:, :, 0], em2[:, :, 1], op=ALU.add
                )
                em_b = (
                    emsum[plo:phi].unsqueeze(2).unsqueeze(3)
                    .broadcast_to([np_, 32, 3, 3])
                )
                nc.vector.tensor_tensor(c14, c14, em_b, op=ALU.add)
            elif part == 2:
                c1 = _CACHE[f"tree{half}b"][1]
                c2 = scr_pool.tile([64, 144], dt.float32, tag="c2",
                                   name=f"c2_{half}")
                _CACHE[f"tree{half}c"] = c2
                _emit_combine(nc, scr_pool, half, 2, 16, c1, c2[:])
            else:
                c2 = _CACHE[f"tree{half}c"]
                out_v = op_st[:, 72 * half : 72 * (half + 1)]
                _emit_combine(nc, scr_pool, half, 3, 8, c2, out_v)

        def emit_numerator(half):
            yoh = cf_sb[:, 54 + 192 * half : 54 + 192 * (half + 1)]
            scr = scr_pool.tile([64, T * UP], dt.float32, tag="nsc",
                                name=f"nsc{half}")
            nc.vector.tensor_tensor(scr[:], e_sb[half][:], yoh, op=ALU.mult)
            nc.vector.tensor_reduce(
                op_st[:, 144 + half : 145 + half], scr[:], axis=AX.X, op=ALU.add
            )

        # ---- stage 1 + interleaved tree emission ----
        for g in range(NGROUP):
            xg = xg_pool.tile([128, HC * 512], dt.float32r, tag="xg")
            dma_eng = nc.sync if g % 2 == 0 else nc.scalar
            dma_eng.dma_start(
                xg[:].rearrange("p (j t) -> p j t", t=512),
                xt_d[:].rearrange("(j p) (b t) -> b p j t", p=128, t=512)[g],
            )
            if g == 0:
                nc.sync.dma_start(wt_sb[:], wt_d[:])
                nc.sync.dma_start(cf_sb[:], cf_d[:])
            e_ps = ps_e_pool.tile([T, 512], dt.float32, tag="eps")
            for j in range(HC):
                nc.tensor.matmul(
                    e_ps[:],
                    wt_sb[:, T * j : T * (j + 1)],
                    xg[:, 512 * j : 512 * (j + 1)],
                    start=(j == 0),
                    stop=(j == HC - 1),
                )
            e_stage = st_pool.tile([T, 512], dt.float32, tag="estage")
            if g % 2 == 0:
                nc.scalar.activation(e_stage[:], e_ps[:], AF.Copy)
            else:
                nc.vector.tensor_copy(e_stage[:], e_ps[:])
            h, r = g // 8, g % 8
            redist_eng = nc.scalar if g % 2 == 0 else nc.sync
            for c in range(T):
                redist_eng.dma_start(
                    e_sb[h][:].rearrange("p (c u) -> p c u", u=UP)[
                        8 * r : 8 * (r + 1), c
                    ],
                    e_stage[c : c + 1, :].rearrange("q (r u) -> q r u", u=UP),
                )
            # interleave first-half tree + numerator; pre-emit sliced parts of
            # the second half's L1 so the tail is only L2/L3
            if g == 8:
                emit_tree(0, 0)
            elif g == 9:
                emit_tree(0, 1)
            elif g == 10:
                emit_tree(0, 2)
            elif g == 11:
                emit_tree(0, 3)
            elif g == 12:
                emit_tree(1, 0, 0, 32)
            elif g == 13:
                emit_numerator(0)
            elif g == 14:
                emit_tree(1, 1, 0, 32)

        # ---- tail: second-half tree remainder + numerator + output ----
        emit_tree(1, 0, 32, 64)
        emit_tree(1, 1, 32, 64)
        emit_tree(1, 2)
        emit_tree(1, 3)
        emit_numerator(1)
        nc.sync.dma_start(op_d[:], op_st[:])

    return nc


def _get_program():
    if "nc" not in _CACHE:
        _CACHE["nc"] = _build_program()
    return _CACHE["nc"]


def kernel(x, y, mask, W, b, start_transitions, end_transitions, transitions):
    x = np.asarray(x, dtype=np.float32)
    y = np.asarray(y, dtype=np.int32)
    W = np.asarray(W, dtype=np.float32)
    b = np.asarray(b, dtype=np.float32)
    start_t = np.asarray(start_transitions, dtype=np.float32)
    end_t = np.asarray(end_transitions, dtype=np.float32)
    trans = np.asarray(transitions, dtype=np.float32)

    nc = _get_program()

    # ---- host-prepared constants ----
    wt = np.zeros((128, HC * T), dtype=np.float32)
    for j in range(HC):
        for c in range(T):
            wt[:, T * j + c] = W[c, 128 * j : 128 * (j + 1)]

    ct = trans + b[None, :]                      # ct[i,j] = trans[i,j]+b[j]
    k1 = np.empty((3, 3, 3), dtype=np.float32)   # k1[i,k,j] = ct[i,j]+ct[j,k]
    k0 = np.empty((3, 3, 3), dtype=np.float32)   # alpha0 row: start[j]+b[j]+ct[j,k]
    sb = start_t + b
    for i in range(3):
        for k in range(3):
            for j in range(3):
                k1[i, k, j] = ct[i, j] + ct[j, k]
                k0[i, k, j] = sb[j] + ct[j, k]
    cf_base = np.zeros((64, 54), dtype=np.float32)
    cf_base[:, 0:27] = k1.reshape(27)[None, :]
    cf_base[:, 27:54] = k1.reshape(27)[None, :]
    cf_base[0, 27:54] = k0.reshape(27)

    in_maps = []
    for core in range(NCORES):
        b0 = BL * core
        yc = y[b0 : b0 + BL].reshape(2, 64, UP)           # (h, p, u)
        yoh = np.zeros((64, 2, T, UP), dtype=np.float32)  # (p, h, c, u)
        for c in range(T):
            yoh[:, :, c, :] = (yc == c).transpose(1, 0, 2)
        cf = np.concatenate([cf_base, yoh.reshape(64, 2 * 192)], axis=1)
        im = {
            "xt": np.ascontiguousarray(x[b0 : b0 + BL].reshape(NT, H).T),
            "wt": wt,
            "cf": np.ascontiguousarray(cf),
        }
        in_maps.append(im)

    _CACHE["last_in_maps"] = in_maps
    res = run_bass_kernel_spmd(nc, in_maps, core_ids=list(range(NCORES)))
    results = res.results

    # ---- host epilogue ----
    chains = np.empty((B, 64 * L3M, 3, 3), dtype=np.float64)
    gsum = np.empty(B, dtype=np.float64)
    for core in range(NCORES):
        op = np.asarray(results[core]["op"], dtype=np.float64)  # [64, 146]
        for h in range(BL):
            bidx = BL * core + h
            chains[bidx] = op[:, 72 * h : 72 * (h + 1)].reshape(64 * L3M, 3, 3)
            gsum[bidx] = op[:, 144 + h].sum()

    # vectorized log-semiring product over the chain (float64)
    cur = chains
    while cur.shape[1] > 1:
        A = cur[:, 0::2]                                   # [B, n, 3, 3] (i,j)
        Bm = cur[:, 1::2]                                  # [B, n, 3, 3] (j,k)
        s = A[:, :, :, :, None] + Bm[:, :, None, :, :]     # [B, n, i, j, k]
        m = s.max(axis=3)
        cur = m + np.log(np.exp(s - m[:, :, :, None, :]).sum(axis=3))
    P = cur[:, 0]                                          # [B, 3, 3]

    losses = np.zeros(B, dtype=np.float64)
    for bidx in range(B):
        yb = y[bidx]
        az = P[bidx, 0, :] + end_t.astype(np.float64)
        mz = az.max()
        denom = mz + np.log(np.exp(az - mz).sum())
        num = (
            start_t[yb[0]]
            + gsum[bidx]
            + b[yb].sum()                     # bias not in device emissions
            + trans[yb[:-1], yb[1:]].sum()
            + end_t[yb[-1]]
        )
        losses[bidx] = num - denom
    return np.float32(-np.mean(losses))


# revision 14
# speedup vs baseline: 1.0303x; 1.0303x over previous
"""BERT-CRF loss kernel for Trainium2 (8 NeuronCores, data-parallel over batch).

Computation: emissions = x @ W.T + b; CRF NLL = mean over batch of
(denominator log-partition - numerator tag-path score).

v3 strategy per core (2 sequences, 8192 time steps):
  Sharding/layout: each core receives its batch shard pre-transposed as
  xT [768, 8192] (h-major), so the h-contraction lands on the partition dim
  directly -- no on-device transposes or casts.

  Stage 1 (memory-bound bulk): 8 big DMAs ([128, 6, 1024] f32 = 3 MB each,
  issued alternately from the SP and ACT queues) stream xT through SBUF;
  per 512-t group, 6 accumulating f32r matmuls produce e[3, 512] in PSUM at
  full moving-rate (f32r streams 1 col/cycle for >=256-col moves); PSUM is
  staged to SBUF and redistributed by 3 small DMAs per group into per-half
  tiles e_sb[h][p, c, u] (partition p holds 64 consecutive time steps).

  Stage 2 (CRF denominator): forward algorithm as a chain of log-semiring
  products of 3x3 matrices M_t[i,j] = trans[i,j] + b[j] + e_t[j].  Each
  partition tree-combines its 64 consecutive matrices in 3 levels (64 -> 8),
  using a host-precomputed K[i,k,j] = ct[i,j] + ct[j,k] table to fuse level 1
  and a per-timestep-max rescaling so level 1 needs no max-reduce.  The half
  covering sequence 0 runs interleaved with stage-1 groups 8-15; only the
  second half's tree is a tail.  The remaining 8 matrices per partition ship
  to the host, which finishes the product per sequence in float64.

  Numerator: e * one-hot(y) multiply + free-dim reduce per half gives
  sum_t e[t, y_t] per partition; host sums and adds start/end/transition/bias
  path scores (tiny O(B*S) int gathers, as in torchcrf's score decomposition).

Assumes mask == all-ones (guaranteed by the problem spec: fill "ones").
"""

import sys

sys.path.insert(0, "/opt/trn_rl_repo")

import numpy as np
from contextlib import ExitStack

import concourse.bass as bass
import concourse.mybir as mybir
import concourse.tile as tile
from concourse.bass_utils import run_bass_kernel_spmd

dt = mybir.dt
AF = mybir.ActivationFunctionType
ALU = mybir.AluOpType
AX = mybir.AxisListType

# ---------------------------------------------------------------------------
# The walrus build in this container accepts at most ONE sync wait per
# instruction.  Legalize the serialized BIR by moving extra waits onto
# preceding same-engine NoOps (each carrying exactly one wait).
# ---------------------------------------------------------------------------
_orig_to_json_bytes = bass.Bass.to_json_bytes


def _legalized_to_json_bytes(self):
    import json as _json

    m = _json.loads(_orig_to_json_bytes(self))
    ctr = 0
    for fn in m.get("functions", []):
        for blk in fn.get("blocks", []):
            insts = blk.get("instructions", [])
            out = []
            for inst in insts:
                si = inst.get("sync_info") or {}
                waits = si.get("on_wait") or []
                if len(waits) > 1:
                    for w in waits[:-1]:
                        ctr += 1
                        out.append(
                            {
                                "debug": inst.get("debug", 0),
                                "engine": inst["engine"],
                                "ins": [],
                                "outs": [],
                                "name": f"lw-{ctr}",
                                "opcode": "NoOp",
                                "sync_info": {"on_update": [], "on_wait": [w]},
                            }
                        )
                    si["on_wait"] = [waits[-1]]
                out.append(inst)
            blk["instructions"] = out
    return _json.dumps(m).encode()


bass.Bass.to_json_bytes = _legalized_to_json_bytes

B, S, H, T = 16, 4096, 768, 3
NCORES = 8
BL = B // NCORES          # sequences per core = 2
NT = BL * S               # 8192 time steps per core
NGROUP = 16               # groups of 512 time steps
NBLOCK = 8                # xT DMA blocks of 2 groups (1024 t)
HC = H // 128             # 6 h-chunks
UP = 64                   # time steps per partition
L3M = 8                   # matrices per partition shipped to host

_CACHE = {}


def _emit_combine(nc, scr, half, lvl, nm, c_in, c_out):
    """Generic log-semiring pair-combine level: c_in [64, 2*nm, 9] (as flat
    [64, 18*nm]) -> c_out view [64, nm*9].  9 ops: 3 S-adds, maxred, sub,
    exp(ACT), addred, ln(ACT), add."""
    s_t = scr.tile([64, nm * 27], dt.float32, tag=f"s{lvl}", name=f"s{lvl}_{half}")
    sub_t = scr.tile([64, nm * 27], dt.float32, tag=f"sb{lvl}", name=f"sb{lvl}_{half}")
    mx_t = scr.tile([64, nm * 9], dt.float32, tag=f"mx{lvl}", name=f"mx{lvl}_{half}")
    sm_t = scr.tile([64, nm * 9], dt.float32, tag=f"sm{lvl}", name=f"sm{lvl}_{half}")
    cv = c_in[:].rearrange("p (m two e) -> p m two e", two=2, e=9)
    b_kj = cv[:, :, 1, :].rearrange("p m (j k) -> p m k j", k=3)          # [p,m,k,j]
    s5 = s_t[:].rearrange("p (m i k j) -> p m i k j", i=3, k=3, j=3)
    for i in range(3):
        a_i = (
            cv[:, :, 0, 3 * i : 3 * i + 3]
            .unsqueeze(2)
            .broadcast_to([64, nm, 3, 3])
        )                                                                  # [p,m,k0,j]
        nc.vector.tensor_tensor(s5[:, :, i, :, :], a_i, b_kj, op=ALU.add)
    s3 = s_t[:].rearrange("p (g j) -> p g j", j=3)
    nc.vector.tensor_reduce(mx_t[:], s3, axis=AX.X, op=ALU.max)
    mx_b = mx_t[:].unsqueeze(2).broadcast_to([64, nm * 9, 3])
    sub3 = sub_t[:].rearrange("p (g j) -> p g j", j=3)
    nc.vector.tensor_tensor(sub3, s3, mx_b, op=ALU.subtract)
    nc.scalar.activation(sub_t[:], sub_t[:], AF.Exp)
    nc.vector.tensor_reduce(sm_t[:], sub3, axis=AX.X, op=ALU.add)
    nc.scalar.activation(sm_t[:], sm_t[:], AF.Ln)
    nc.vector.tensor_tensor(c_out, sm_t[:], mx_t[:], op=ALU.add)


def _build_program():
    nc = bass.Bass()
    tc = tile.TileContext(nc)

    # ---- DRAM I/O ----
    xt_d = nc.dram_tensor("xt", [H, NT], dt.float32r, kind="ExternalInput")
    wt_d = nc.dram_tensor("wt", [128, HC * T], dt.float32r, kind="ExternalInput")
    cf_d = nc.dram_tensor("cf", [64, 54 + 2 * 192], dt.float32, kind="ExternalInput")
    op_d = nc.dram_tensor("op", [64, 2 * L3M * 9 + 2], dt.float32, kind="ExternalOutput")

    with tc, ExitStack() as ctx:
        const_pool = ctx.enter_context(tc.tile_pool(name="const", bufs=1))
        xg_pool = ctx.enter_context(tc.tile_pool(name="xg", bufs=2))
        st_pool = ctx.enter_context(tc.tile_pool(name="st", bufs=2))
        e_pool = ctx.enter_context(tc.tile_pool(name="e", bufs=1))
        scr_pool = ctx.enter_context(tc.tile_pool(name="scr", bufs=1))
        ps_e_pool = ctx.enter_context(tc.tile_pool(name="pse", bufs=2, space="PSUM"))

        # ---- constants (issued after the first xT block DMA) ----
        wt_sb = const_pool.tile([128, HC * T], dt.float32r, tag="wt")
        cf_sb = const_pool.tile([64, 54 + 2 * 192], dt.float32, tag="cf")
        k1_v = cf_sb[:, 0:27].rearrange("p (ik j) -> p ik j", j=3)
        k0_v = cf_sb[:, 27:54].rearrange("p (ik j) -> p ik j", j=3)

        # per-half emission tiles: e_sb[h][p, c, u], partition p holds
        # 64 consecutive time steps of sequence h
        e_sb = [
            e_pool.tile([64, T * UP], dt.float32, tag=f"e{h}", name=f"e{h}")
            for h in range(2)
        ]
        # outputs staging: 2*72 tree results + 2 numerator columns
        op_st = e_pool.tile([64, 2 * L3M * 9 + 2], dt.float32, tag="opst")

        def emit_tree(half, part, plo=0, phi=64):
            """Emit one chunk of the in-partition tree for one half.
            part 0: rescale prep + L1 S-build; part 1: L1 finish;
            part 2: L2; part 3: L3 + write into op_st.  Parts 0/1 may be
            emitted for a partition slice [plo:phi] to overlap stage 1."""
            np_ = phi - plo
            e3 = e_sb[half][:].rearrange("p (c u) -> p c u", u=UP)[plo:phi]
            if part == 0:
                if plo == 0:
                    _CACHE[f"tree{half}"] = (
                        scr_pool.tile([64, UP], dt.float32, tag="emax",
                                      name=f"emax{half}"),
                        scr_pool.tile([64, T * UP], dt.float32, tag="es",
                                      name=f"es{half}"),
                        scr_pool.tile([64, 32 * 27], dt.float32, tag="s1",
                                      name=f"s1_{half}"),
                    )
                emax, es_t, s1 = _CACHE[f"tree{half}"]
                emx = emax[plo:phi]
                # emax[p,u] = max_c e[p,c,u];  es = e - emax (range <= 0)
                nc.vector.tensor_tensor(
                    emx, e3[:, 0, :], e3[:, 1, :], op=ALU.max
                )
                nc.vector.tensor_tensor(
                    emx, emx, e3[:, 2, :], op=ALU.max
                )
                emax_b = emx.unsqueeze(1).broadcast_to([np_, T, UP])
                nc.vector.tensor_tensor(
                    es_t[:].rearrange("p (c u) -> p c u", u=UP)[plo:phi], e3,
                    emax_b, op=ALU.subtract,
                )
                # L1 S-build: S[p,m,ik,j] = K[ik,j] + esA[m,j]
                es3 = es_t[:].rearrange("p (c u) -> p c u", u=UP)[plo:phi]
                esA = es3.rearrange("p c (m two) -> p m two c", two=2)
                s4 = s1[:].rearrange("p (m ik j) -> p m ik j", ik=9, j=3)[plo:phi]
                a1 = esA[:, 1:, 0, :].unsqueeze(2).broadcast_to([np_, 31, 9, 3])
                nc.vector.tensor_tensor(
                    s4[:, 1:, :, :],
                    k1_v[plo:phi].unsqueeze(1).broadcast_to([np_, 31, 9, 3]),
                    a1, op=ALU.add,
                )
                a0 = esA[:, 0:1, 0, :].unsqueeze(2).broadcast_to([np_, 1, 9, 3])
                nc.vector.tensor_tensor(
                    s4[:, 0:1, :, :],
                    k0_v[plo:phi].unsqueeze(1).broadcast_to([np_, 1, 9, 3]),
                    a0, op=ALU.add,
                )
            elif part == 1:
                emax, es_t, s1 = _CACHE[f"tree{half}"]
                es3 = es_t[:].rearrange("p (c u) -> p c u", u=UP)[plo:phi]
                if plo == 0:
                    _CACHE[f"tree{half}b"] = (
                        scr_pool.tile([64, 288], dt.float32, tag="sm1",
                                      name=f"sm1_{half}"),
                        scr_pool.tile([64, 288], dt.float32, tag="c1",
                                      name=f"c1_{half}"),
                        scr_pool.tile([64, 32], dt.float32, tag="ems",
                                      name=f"ems{half}"),
                    )
                sm1, c1, emsum = _CACHE[f"tree{half}b"]
                nc.scalar.activation(s1[plo:phi], s1[plo:phi], AF.Exp)
                nc.vector.tensor_reduce(
                    sm1[plo:phi],
                    s1[:].rearrange("p (g j) -> p g j", j=3)[plo:phi],
                    axis=AX.X, op=ALU.add,
                )
                nc.scalar.activation(sm1[plo:phi], sm1[plo:phi], AF.Ln)
                # C1 = ln-sum + esB[k] + (emaxA + emaxB)  (= lnsum + eB + emaxA)
                esB = (
                    es3.rearrange("p c (m two) -> p m two c", two=2)[:, :, 1, :]
                    .unsqueeze(2)
                    .broadcast_to([np_, 32, 3, 3])
                )                                                  # [p,m,i0,k]
                c14 = c1[:].rearrange("p (m i k) -> p m i k", i=3, k=3)[plo:phi]
                nc.vector.tensor_tensor(
                    c14,
                    sm1[:].rearrange("p (m i k) -> p m i k", i=3, k=3)[plo:phi],
                    esB, op=ALU.add,
                )
                em2 = emax[:].rearrange("p (m two) -> p m two", two=2)[plo:phi]
                nc.vector.tensor_tensor(
                    emsum[plo:phi], em2[:, :, 0], em2[:, :, 1], op=ALU.add
                )
                em_b = (
                    emsum[plo:phi].unsqueeze(2).unsqueeze(3)
                    .broadcast_to([np_, 32, 3, 3])
                )
                nc.vector.tensor_tensor(c14, c14, em_b, op=ALU.add)
            elif part == 2:
                c1 = _CACHE[f"tree{half}b"][1]
                c2 = scr_pool.tile([64, 144], dt.float32, tag="c2",
                                   name=f"c2_{half}")
                _CACHE[f"tree{half}c"] = c2
                _emit_combine(nc, scr_pool, half, 2, 16, c1, c2[:])
            else:
                c2 = _CACHE[f"tree{half}c"]
                out_v = op_st[:, 72 * half : 72 * (half + 1)]
                _emit_combine(nc, scr_pool, half, 3, 8, c2, out_v)

        def emit_numerator(half):
            yoh = cf_sb[:, 54 + 192 * half : 54 + 192 * (half + 1)]
            scr = scr_pool.tile([64, T * UP], dt.float32, tag="nsc",
                                name=f"nsc{half}")
            nc.vector.tensor_tensor(scr[:], e_sb[half][:], yoh, op=ALU.mult)
            nc.vector.tensor_reduce(
                op_st[:, 144 + half : 145 + half], scr[:], axis=AX.X, op=ALU.add
            )

        # ---- stage 1 + interleaved tree emission ----
        for g in range(NGROUP):
            xg = xg_pool.tile([128, HC * 512], dt.float32r, tag="xg")
            nc.sync.dma_start(
                xg[:].rearrange("p (j t) -> p j t", t=512),
                xt_d[:].rearrange("(j p) (b t) -> b p j t", p=128, t=512)[g],
            )
            if g == 0:
                nc.sync.dma_start(wt_sb[:], wt_d[:])
                nc.sync.dma_start(cf_sb[:], cf_d[:])
            e_ps = ps_e_pool.tile([T, 512], dt.float32, tag="eps")
            for j in range(HC):
                nc.tensor.matmul(
                    e_ps[:],
                    wt_sb[:, T * j : T * (j + 1)],
                    xg[:, 512 * j : 512 * (j + 1)],
                    start=(j == 0),
                    stop=(j == HC - 1),
                )
            e_stage = st_pool.tile([T, 512], dt.float32, tag="estage")
            if g % 2 == 0:
                nc.scalar.activation(e_stage[:], e_ps[:], AF.Copy)
            else:
                nc.vector.tensor_copy(e_stage[:], e_ps[:])
            h, r = g // 8, g % 8
            for c in range(T):
                nc.scalar.dma_start(
                    e_sb[h][:].rearrange("p (c u) -> p c u", u=UP)[
                        8 * r : 8 * (r + 1), c
                    ],
                    e_stage[c : c + 1, :].rearrange("q (r u) -> q r u", u=UP),
                )
            # interleave first-half tree + numerator; pre-emit sliced parts of
            # the second half's L1 so the tail is only L2/L3
            if g == 8:
                emit_tree(0, 0)
            elif g == 9:
                emit_tree(0, 1)
            elif g == 10:
                emit_tree(0, 2)
            elif g == 11:
                emit_tree(0, 3)
            elif g == 12:
                emit_tree(1, 0, 0, 32)
            elif g == 13:
                emit_numerator(0)
            elif g == 14:
                emit_tree(1, 1, 0, 32)

        # ---- tail: second-half tree remainder + numerator + output ----
        emit_tree(1, 0, 32, 64)
        emit_tree(1, 1, 32, 64)
        emit_tree(1, 2)
        emit_tree(1, 3)
        emit_numerator(1)
        nc.sync.dma_start(op_d[:], op_st[:])

    return nc


def _get_program():
    if "nc" not in _CACHE:
        _CACHE["nc"] = _build_program()
    return _CACHE["nc"]


def kernel(x, y, mask, W, b, start_transitions, end_transitions, transitions):
    x = np.asarray(x, dtype=np.float32)
    y = np.asarray(y, dtype=np.int32)
    W = np.asarray(W, dtype=np.float32)
    b = np.asarray(b, dtype=np.float32)
    start_t = np.asarray(start_transitions, dtype=np.float32)
    end_t = np.asarray(end_transitions, dtype=np.float32)
    trans = np.asarray(transitions, dtype=np.float32)

    nc = _get_program()

    # ---- host-prepared constants ----
    wt = np.zeros((128, HC * T), dtype=np.float32)
    for j in range(HC):
        for c in range(T):
            wt[:, T * j + c] = W[c, 128 * j : 128 * (j + 1)]

    ct = trans + b[None, :]                      # ct[i,j] = trans[i,j]+b[j]
    k1 = np.empty((3, 3, 3), dtype=np.float32)   # k1[i,k,j] = ct[i,j]+ct[j,k]
    k0 = np.empty((3, 3, 3), dtype=np.float32)   # alpha0 row: start[j]+b[j]+ct[j,k]
    sb = start_t + b
    for i in range(3):
        for k in range(3):
            for j in range(3):
                k1[i, k, j] = ct[i, j] + ct[j, k]
                k0[i, k, j] = sb[j] + ct[j, k]
    cf_base = np.zeros((64, 54), dtype=np.float32)
    cf_base[:, 0:27] = k1.reshape(27)[None, :]
    cf_base[:, 27:54] = k1.reshape(27)[None, :]
    cf_base[0, 27:54] = k0.reshape(27)

    in_maps = []
    for core in range(NCORES):
        b0 = BL * core
        yc = y[b0 : b0 + BL].reshape(2, 64, UP)           # (h, p, u)
        yoh = np.zeros((64, 2, T, UP), dtype=np.float32)  # (p, h, c, u)
        for c in range(T):
            yoh[:, :, c, :] = (yc == c).transpose(1, 0, 2)
        cf = np.concatenate([cf_base, yoh.reshape(64, 2 * 192)], axis=1)
        im = {
            "xt": np.ascontiguousarray(x[b0 : b0 + BL].reshape(NT, H).T),
            "wt": wt,
            "cf": np.ascontiguousarray(cf),
        }
        in_maps.append(im)

    _CACHE["last_in_maps"] = in_maps
    res = run_bass_kernel_spmd(nc, in_maps, core_ids=list(range(NCORES)))
    results = res.results

    # ---- host epilogue ----
    chains = np.empty((B, 64 * L3M, 3, 3), dtype=np.float64)
    gsum = np.empty(B, dtype=np.float64)
    for core in range(NCORES):
        op = np.asarray(results[core]["op"], dtype=np.float64)  # [64, 146]
        for h in range(BL):
            bidx = BL * core + h
            chains[bidx] = op[:, 72 * h : 72 * (h + 1)].reshape(64 * L3M, 3, 3)
            gsum[bidx] = op[:, 144 + h].sum()

    # vectorized log-semiring product over the chain (float64)
    cur = chains
    while cur.shape[1] > 1:
        A = cur[:, 0::2]                                   # [B, n, 3, 3] (i,j)
        Bm = cur[:, 1::2]                                  # [B, n, 3, 3] (j,k)
        s = A[:, :, :, :, None] + Bm[:, :, None, :, :]     # [B, n, i, j, k]
        m = s.max(axis=3)
        cur = m + np.log(np.exp(s - m[:, :, :, None, :]).sum(axis=3))
    P = cur[:, 0]                                          # [B, 3, 3]

    losses = np.zeros(B, dtype=np.float64)
    for bidx in range(B):
        yb = y[bidx]
        az = P[bidx, 0, :] + end_t.astype(np.float64)
        mz = az.max()
        denom = mz + np.log(np.exp(az - mz).sum())
        num = (
            start_t[yb[0]]
            + gsum[bidx]
            + b[yb].sum()                     # bias not in device emissions
            + trans[yb[:-1], yb[1:]].sum()
            + end_t[yb[-1]]
        )
        losses[bidx] = num - denom
    return np.float32(-np.mean(losses))
